# revision 6
# baseline (speedup 1.0000x reference)
"""Linear (feature-map) attention for Trainium2, 8-core head-parallel, bf16.

Math per (b,h), s = D**-0.25:
    phi(x) = elu(s*x) + 1  ==  min(exp(s*x), 1) + max(s*x, 0)
    kv     = phi_k^T @ [v | 1]            # [64, 65]; col 64 = sum_s phi_k
    out    = (phi_q @ kv[:, :64]) / (phi_q @ kv[:, 64])
(The reference's +1e-8 in the denominator is far below fp32 ulp of the
normalizer; the attention mask is all-ones per the input spec.)

All on-chip compute in bf16 (PE matmuls run 4x faster than fp32; rel-err
budget 2e-2 >> bf16 rounding). Inputs are cast fp32->bf16 during the DMA
load (SWDGE); output is computed fp32 from the fp32 PSUM accumulators.

Per core: 8 of the 64 (b,h) slices, processed as 4 pairs of heads.
SBUF s-layout: s = 32*p + t (p = partition, t = 0..31); every load moves
128 partitions x 4KB contiguous per head.

Engine plan (per-rep busy estimates from the instruction cost model):
  PE  : q-transpose per head via transpose-mode matmul (bf16) -> [dA|dB, s]
        mm1 col-sliced per head from interleaved phi_k (lhsT contiguous 128)
        mm2 full-K against block-diagonal kv -> [128s, 130] per s-tile
  ACT : Exp on k and q, Relu on q (Exp+Relu share one ACT table)
  Pool: Relu on k (tensor_scalar max+mult), SWDGE cast-DMA issue
  DVE : stt (min+add) assembling phi, kvbd assembly, reciprocal, normalize
"""

import numpy as np

B, H, S_FULL, D = 4, 16, 4096, 64
N_CORES = 8
BH = B * H
BH_PER_CORE = BH // N_CORES  # 8
P = 128

SCALE = float(D) ** -0.25          # 0.3535533905932738

_NC_CACHE = {}


def _patch_tile_drain():
    """The walrus build in this container accepts at most ONE sync wait per
    instruction, but TileContext's kernel-tail drain aggregates every
    outstanding semaphore onto a single SP Drain. Replace it with one
    single-wait SP nop per semaphore followed by the drain; likewise split
    any scheduled instruction that ends up with more than one sync wait."""
    import concourse.mybir as mybir
    import concourse.tile as tile
    from concourse.vector_clock import ScopedClock

    if getattr(tile.TileContext, "_single_wait_drain_patch", False):
        return

    def _drain_and_barrier(self, tick_clock, wait_clock):
        collector = self.nc.sync.nop()
        wait_clock.add_sem_waits(
            collector.ins, ScopedClock({None: tick_clock.global_clock})
        )
        waits = list(collector.ins.sync_info.on_wait) if collector.ins.sync_info else []
        collector.ins.sync_info = mybir.SyncInfo(on_wait=waits[:1], on_update=[])
        for w in waits[1:]:
            nop = self.nc.sync.nop()
            nop.ins.sync_info = mybir.SyncInfo(on_wait=[w], on_update=[])
        self.nc.sync.drain()
        self.nc.all_engine_barrier()
        assert self.sems is not None
        popped = self.nc._tile_sem_poison_stack.pop()
        assert popped is self._sem_poison
        self.nc.clear_and_free_semaphores(list(self.sems.allocated().values()))
        self.nc.all_engine_barrier()

    tile.TileContext._drain_and_barrier = _drain_and_barrier

    _orig_commit = tile.TileContext._commit_instruction

    def _commit_instruction(self, inst, lazy_reg_writes=True):
        si = getattr(inst, "sync_info", None)
        if si is not None and si.on_wait and len(si.on_wait) > 1:
            waits = list(si.on_wait)
            for w in waits[:-1]:
                nop = mybir.InstNoOp(
                    name=self.nc.get_next_instruction_name(),
                    engine=inst.engine,
                    text_hint="wait_split",
                    bass_nofuse=True,
                )
                nop.sync_info = mybir.SyncInfo(on_wait=[w], on_update=[])
                _orig_commit(self, nop, lazy_reg_writes)
            inst.sync_info = mybir.SyncInfo(
                on_wait=[waits[-1]], on_update=list(si.on_update or [])
            )
        return _orig_commit(self, inst, lazy_reg_writes)

    tile.TileContext._commit_instruction = _commit_instruction
    tile.TileContext._single_wait_drain_patch = True


def build_bass(n_heads=BH_PER_CORE, S=S_FULL, n_reps=1):
    import concourse.bass as bass
    import concourse.mybir as mybir
    import concourse.tile as tile

    _patch_tile_drain()

    f32 = mybir.dt.float32
    nc = bass.Bass("TRN2")
    q_d = nc.dram_tensor("q", [n_heads, S, D], f32, kind="ExternalInput")
    k_d = nc.dram_tensor("k", [n_heads, S, D], f32, kind="ExternalInput")
    v_d = nc.dram_tensor("v", [n_heads, S, D], f32, kind="ExternalInput")
    o_d = nc.dram_tensor(
        "out", [n_heads, S, D], mybir.dt.bfloat16, kind="ExternalOutput"
    )
    with tile.TileContext(nc) as tc:
        _emit(tc, q_d, k_d, v_d, o_d, n_heads, S, n_reps)
    nc.finalize()
    return nc


def _emit(tc, q_d, k_d, v_d, o_d, n_heads, S, n_reps=1):
    from contextlib import ExitStack

    import concourse.mybir as mybir
    from concourse.masks import make_identity

    nc = tc.nc
    f32 = mybir.dt.float32
    bf16 = mybir.dt.bfloat16
    Alu = mybir.AluOpType
    Act = mybir.ActivationFunctionType

    T = S // P                # s-tiles per head (32 for S=4096)
    n_pairs = n_heads // 2

    ctx = ExitStack()
    with ctx:
        p_const = ctx.enter_context(tc.tile_pool(name="const", bufs=1))
        p_qin = ctx.enter_context(tc.tile_pool(name="qin", bufs=3))
        p_kin = ctx.enter_context(tc.tile_pool(name="kin", bufs=3))
        p_vin = ctx.enter_context(tc.tile_pool(name="vin", bufs=3))
        p_ek = ctx.enter_context(tc.tile_pool(name="ek", bufs=2))
        p_rk = ctx.enter_context(tc.tile_pool(name="rk", bufs=2))
        p_eq = ctx.enter_context(tc.tile_pool(name="eq", bufs=2))
        p_phiqt = ctx.enter_context(tc.tile_pool(name="phiqt", bufs=2))
        p_small = ctx.enter_context(tc.tile_pool(name="small", bufs=2))
        p_out = ctx.enter_context(tc.tile_pool(name="outb", bufs=2))
        ps_qt = ctx.enter_context(tc.tile_pool(name="psqt", bufs=2, space="PSUM"))
        ps_kva = ctx.enter_context(tc.tile_pool(name="pskva", bufs=1, space="PSUM"))
        ps_kvb = ctx.enter_context(tc.tile_pool(name="pskvb", bufs=1, space="PSUM"))
        ps_kv1 = ctx.enter_context(tc.tile_pool(name="pskv1", bufs=1, space="PSUM"))
        ps_o = ctx.enter_context(tc.tile_pool(name="pso", bufs=2, space="PSUM"))

        ident = p_const.tile([P, P], bf16, tag="ident")
        make_identity(nc, ident[:])
        ones = p_const.tile([P, 1], bf16, tag="ones")
        nc.vector.memset(ones[:], 1.0)

        for _rep in range(n_reps):
            for pr in range(n_pairs):
                _emit_pair(
                    nc, mybir, f32, bf16, Alu, Act, T, pr,
                    p_qin, p_kin, p_vin, p_ek, p_rk, p_eq, p_phiqt, p_small,
                    p_out, ps_qt, ps_kva, ps_kvb, ps_kv1, ps_o,
                    q_d, k_d, v_d, o_d, ident, ones,
                )


def _emit_pair(
    nc, mybir, f32, bf16, Alu, Act, T, pr,
    p_qin, p_kin, p_vin, p_ek, p_rk, p_eq, p_phiqt, p_small, p_out,
    ps_qt, ps_kva, ps_kvb, ps_kv1, ps_o, q_d, k_d, v_d, o_d, ident, ones,
):
    iA, iB = 2 * pr, 2 * pr + 1
    D_ = 64

    # ---- loads: s = 32*p + t layout, SWDGE cast fp32 -> bf16 -------------
    q2 = p_qin.tile([P, 2, T, D_], bf16, tag="q2")
    k2 = p_kin.tile([P, 2, T, D_], f32, tag="k2")
    v2 = p_vin.tile([P, 2, T, D_], bf16, tag="v2")
    for h, i in ((0, iA), (1, iB)):
        nc.gpsimd.dma_start(q2[:, h], q_d[i].rearrange("(p t) d -> p t d", p=P))
        nc.sync.dma_start(k2[:, h], k_d[i].rearrange("(p t) d -> p t d", p=P))
        nc.gpsimd.dma_start(v2[:, h], v_d[i].rearrange("(p t) d -> p t d", p=P))

    # ---- k path: phi_k computed into the (t, h, d)-interleaved layout ----
    # ek[p, t, h, d]: (h, d) contiguous 128 so mm1's lhsT is a single free dim
    ek = p_ek.tile([P, T, 2, D_], bf16, tag="ek")
    rk = p_rk.tile([P, T, 2, D_], bf16, tag="rk")
    kch = max(T // 4, 1)
    for c0 in range(0, T, kch):
        sl = slice(c0, c0 + kch)
        kin = k2[:, :, sl, :].rearrange("p h t d -> p t h d")
        nc.scalar.activation(ek[:, sl], kin, Act.Exp, scale=SCALE)
        nc.vector.tensor_scalar(rk[:, sl], kin, 0.0, SCALE, Alu.max, Alu.mult)
        # phi_k = min(exp, 1) + relu, in place
        nc.vector.scalar_tensor_tensor(
            ek[:, sl], ek[:, sl], 1.0, rk[:, sl], Alu.min, Alu.add
        )

    # ---- q path: per-head PE transpose (bf16) -> phi_q transposed --------
    # phiqT[:, j, :]: partitions = (dA | dB), free = the 128 s of tile j
    phiqT = p_phiqt.tile([P, T, P], bf16, tag="phiqt")
    n_qb = (T + 3) // 4
    for jb in range(n_qb):
        njs = min(4, T - 4 * jb)
        qtp = ps_qt.tile([P, 4, P], bf16, tag="qtp")
        for jj in range(njs):
            j = 4 * jb + jj
            for h in (0, 1):
                nc.tensor.matmul(
                    qtp[64 * h : 64 * h + 64, jj, :],
                    q2[:, h, j, :],
                    ident[:],
                    is_transpose=True,
                )
        eq = p_eq.tile([P, 4, P], bf16, tag="eq")
        rq = p_eq.tile([P, 4, P], bf16, tag="rq")
        nc.scalar.activation(eq[:, :njs, :], qtp[:, :njs, :], Act.Exp, scale=SCALE)
        nc.scalar.activation(rq[:, :njs, :], qtp[:, :njs, :], Act.Relu, scale=SCALE)
        nc.vector.scalar_tensor_tensor(
            phiqT[:, 4 * jb : 4 * jb + njs, :],
            eq[:, :njs, :], 1.0, rq[:, :njs, :], Alu.min, Alu.add,
        )

    # ---- mm1: kv = phi_k^T @ v per head, k_one = phi_k^T @ 1 -------------
    # lhsT = ek[:, j] (both heads' (h,d) on its free dim -> out rows (h,d));
    # head h's kv block sits at out rows 64h..64h+63. Each accumulation
    # group owns a whole PSUM bank.
    kva = ps_kva.tile([P, D_], f32, tag="kva", name="kva")
    kvb = ps_kvb.tile([P, D_], f32, tag="kvb", name="kvb")
    kv1 = ps_kv1.tile([P, 1], f32, tag="kv1", name="kv1")
    for j in range(T):
        sta, sp = (j == 0), (j == T - 1)
        lhs = ek[:, j].rearrange("p h d -> p (h d)")
        nc.tensor.matmul(kva[:], lhs, v2[:, 0, j, :], start=sta, stop=sp)
        nc.tensor.matmul(kvb[:], lhs, v2[:, 1, j, :], start=sta, stop=sp)
        nc.tensor.matmul(kv1[:], lhs, ones[:], start=sta, stop=sp)

    # block-diagonal [128, 130]: rows 0-63 -> cols 0-64 (head A),
    # rows 64-127 -> cols 65-129 (head B); zeros elsewhere
    kvbd = p_small.tile([P, 130], bf16, tag="kvbd")
    nc.vector.memset(kvbd[:], 0.0)
    nc.vector.tensor_copy(out=kvbd[0:64, 0:64], in_=kva[0:64, :])
    nc.vector.tensor_copy(out=kvbd[0:64, 64:65], in_=kv1[0:64, :])
    nc.vector.tensor_copy(out=kvbd[64:128, 65:129], in_=kvb[64:128, :])
    nc.vector.tensor_copy(out=kvbd[64:128, 129:130], in_=kv1[64:128, :])

    # ---- mm2 + fused normalize ------------------------------------------
    out2 = p_out.tile([P, 2, T, D_], bf16, tag="out2")
    rc = p_small.tile([P, 2, T], f32, tag="recip")
    n_ob = (T + 2) // 3
    for m in range(n_ob):
        w = min(3, T - 3 * m)
        op = ps_o.tile([P, 3, 130], f32, tag="op")
        for jj in range(w):
            j = 3 * m + jj
            # [128s, 130]: cols 0-64 head A (col 64 = norm), 65-129 head B
            nc.tensor.matmul(op[:, jj, :], phiqT[:, j, :], kvbd[:])
        opv = op[:, 0:w, :].rearrange("p j (h e) -> p j h e", h=2)
        nc.vector.reciprocal(
            rc[:, :, 3 * m : 3 * m + w],
            opv[:, :, :, 64].rearrange("p j h -> p h j"),
        )
        nc.vector.tensor_tensor(
            out2[:, :, 3 * m : 3 * m + w, :],
            opv[:, :, :, 0:64].rearrange("p j h e -> p h j e"),
            rc[:, :, 3 * m : 3 * m + w, None].to_broadcast((P, 2, w, D_)),
            Alu.mult,
        )

    # ---- store ----
    for h, i in ((0, iA), (1, iB)):
        od = o_d[i].rearrange("(p t) d -> p t d", p=P)
        nc.sync.dma_start(od, out2[:, h])


def _get_nc():
    key = (BH_PER_CORE, S_FULL)
    if key not in _NC_CACHE:
        _NC_CACHE[key] = build_bass(*key)
    return _NC_CACHE[key]


def run_sharded(q, k, v, trace=False):
    """q/k/v: [BH, S, D] fp32 numpy. Returns ([BH, S, D] fp32, BassKernelResults)."""
    from concourse.bass_utils import run_bass_kernel_spmd

    nc = _get_nc()
    in_maps = []
    for c in range(N_CORES):
        sl = slice(c * BH_PER_CORE, (c + 1) * BH_PER_CORE)
        in_maps.append(
            {
                "q": np.ascontiguousarray(q[sl]),
                "k": np.ascontiguousarray(k[sl]),
                "v": np.ascontiguousarray(v[sl]),
            }
        )
    res = run_bass_kernel_spmd(
        nc, in_maps, core_ids=list(range(N_CORES)), trace=trace
    )
    out = np.concatenate(
        [np.asarray(r["out"], dtype=np.float32) for r in res.results], axis=0
    )
    return out, res


def kernel(query, key, value, attention_mask=None):
    q = np.asarray(query, dtype=np.float32).reshape(BH, S_FULL, D)
    k = np.asarray(key, dtype=np.float32).reshape(BH, S_FULL, D)
    v = np.asarray(value, dtype=np.float32).reshape(BH, S_FULL, D)
    out, _ = run_sharded(q, k, v, trace=False)
    return out.reshape(B, H, S_FULL, D)


# revision 7
# speedup vs baseline: 1.2321x; 1.2321x over previous
"""Linear (feature-map) attention for Trainium2, 8-core head-parallel, bf16.

Math per (b,h), s = D**-0.25:
    phi(x) = elu(s*x) + 1  ==  min(exp(s*x), 1) + max(s*x, 0)
    kv     = phi_k^T @ [v | 1]            # [64, 65]; col 64 = sum_s phi_k
    out    = (phi_q @ kv[:, :64]) / (phi_q @ kv[:, 64])
(The reference's +1e-8 in the denominator is far below fp32 ulp of the
normalizer; the attention mask is all-ones per the input spec.)

All on-chip compute in bf16 (PE matmuls run 4x faster than fp32; rel-err
budget 2e-2 >> bf16 rounding). Inputs are cast fp32->bf16 during the DMA
load (SWDGE); output is computed fp32 from the fp32 PSUM accumulators.

Per core: 8 of the 64 (b,h) slices, processed as 4 pairs of heads.
SBUF s-layout: s = 32*p + t (p = partition, t = 0..31); every load moves
128 partitions x 4KB contiguous per head.

Engine plan (per-rep busy estimates from the instruction cost model):
  PE  : q-transpose per head via transpose-mode matmul (bf16) -> [dA|dB, s]
        mm1 col-sliced per head from interleaved phi_k (lhsT contiguous 128)
        mm2 full-K against block-diagonal kv -> [128s, 130] per s-tile
  ACT : Exp on k and q, Relu on q (Exp+Relu share one ACT table)
  Pool: Relu on k (tensor_scalar max+mult), SWDGE cast-DMA issue
  DVE : stt (min+add) assembling phi, kvbd assembly, reciprocal, normalize
"""

import numpy as np

B, H, S_FULL, D = 4, 16, 4096, 64
N_CORES = 8
BH = B * H
BH_PER_CORE = BH // N_CORES  # 8
P = 128

SCALE = float(D) ** -0.25          # 0.3535533905932738

_NC_CACHE = {}


def _patch_tile_drain():
    """The walrus build in this container accepts at most ONE sync wait per
    instruction, but TileContext's kernel-tail drain aggregates every
    outstanding semaphore onto a single SP Drain. Replace it with one
    single-wait SP nop per semaphore followed by the drain; likewise split
    any scheduled instruction that ends up with more than one sync wait."""
    import concourse.mybir as mybir
    import concourse.tile as tile
    from concourse.vector_clock import ScopedClock

    if getattr(tile.TileContext, "_single_wait_drain_patch", False):
        return

    def _drain_and_barrier(self, tick_clock, wait_clock):
        collector = self.nc.sync.nop()
        wait_clock.add_sem_waits(
            collector.ins, ScopedClock({None: tick_clock.global_clock})
        )
        waits = list(collector.ins.sync_info.on_wait) if collector.ins.sync_info else []
        collector.ins.sync_info = mybir.SyncInfo(on_wait=waits[:1], on_update=[])
        for w in waits[1:]:
            nop = self.nc.sync.nop()
            nop.ins.sync_info = mybir.SyncInfo(on_wait=[w], on_update=[])
        self.nc.sync.drain()
        self.nc.all_engine_barrier()
        assert self.sems is not None
        popped = self.nc._tile_sem_poison_stack.pop()
        assert popped is self._sem_poison
        self.nc.clear_and_free_semaphores(list(self.sems.allocated().values()))
        self.nc.all_engine_barrier()

    tile.TileContext._drain_and_barrier = _drain_and_barrier

    _orig_commit = tile.TileContext._commit_instruction

    def _commit_instruction(self, inst, lazy_reg_writes=True):
        si = getattr(inst, "sync_info", None)
        if si is not None and si.on_wait and len(si.on_wait) > 1:
            waits = list(si.on_wait)
            for w in waits[:-1]:
                nop = mybir.InstNoOp(
                    name=self.nc.get_next_instruction_name(),
                    engine=inst.engine,
                    text_hint="wait_split",
                    bass_nofuse=True,
                )
                nop.sync_info = mybir.SyncInfo(on_wait=[w], on_update=[])
                _orig_commit(self, nop, lazy_reg_writes)
            inst.sync_info = mybir.SyncInfo(
                on_wait=[waits[-1]], on_update=list(si.on_update or [])
            )
        return _orig_commit(self, inst, lazy_reg_writes)

    tile.TileContext._commit_instruction = _commit_instruction
    tile.TileContext._single_wait_drain_patch = True


def build_bass(n_heads=BH_PER_CORE, S=S_FULL, n_reps=1):
    import concourse.bass as bass
    import concourse.mybir as mybir
    import concourse.tile as tile

    _patch_tile_drain()

    f32 = mybir.dt.float32
    nc = bass.Bass("TRN2")
    q_d = nc.dram_tensor("q", [n_heads, S, D], f32, kind="ExternalInput")
    k_d = nc.dram_tensor("k", [n_heads, S, D], f32, kind="ExternalInput")
    v_d = nc.dram_tensor("v", [n_heads, S, D], f32, kind="ExternalInput")
    o_d = nc.dram_tensor(
        "out", [n_heads, S, D], mybir.dt.bfloat16, kind="ExternalOutput"
    )
    with tile.TileContext(nc) as tc:
        _emit(tc, q_d, k_d, v_d, o_d, n_heads, S, n_reps)
    nc.finalize()
    return nc


def _emit(tc, q_d, k_d, v_d, o_d, n_heads, S, n_reps=1):
    from contextlib import ExitStack

    import concourse.mybir as mybir
    from concourse.masks import make_identity

    nc = tc.nc
    f32 = mybir.dt.float32
    bf16 = mybir.dt.bfloat16
    Alu = mybir.AluOpType
    Act = mybir.ActivationFunctionType

    T = S // P                # s-tiles per head (32 for S=4096)
    n_pairs = n_heads // 2

    ctx = ExitStack()
    with ctx:
        p_const = ctx.enter_context(tc.tile_pool(name="const", bufs=1))
        p_qin = ctx.enter_context(tc.tile_pool(name="qin", bufs=2))
        p_kin = ctx.enter_context(tc.tile_pool(name="kin", bufs=2))
        p_vin = ctx.enter_context(tc.tile_pool(name="vin", bufs=2))
        p_ek = ctx.enter_context(tc.tile_pool(name="ek", bufs=2))
        p_rk = ctx.enter_context(tc.tile_pool(name="rk", bufs=2))
        p_eq = ctx.enter_context(tc.tile_pool(name="eq", bufs=2))
        p_phiqt = ctx.enter_context(tc.tile_pool(name="phiqt", bufs=2))
        p_small = ctx.enter_context(tc.tile_pool(name="small", bufs=2))
        p_out = ctx.enter_context(tc.tile_pool(name="outb", bufs=2))
        ps_qt = ctx.enter_context(tc.tile_pool(name="psqt", bufs=2, space="PSUM"))
        ps_kva = ctx.enter_context(tc.tile_pool(name="pskva", bufs=1, space="PSUM"))
        ps_kvb = ctx.enter_context(tc.tile_pool(name="pskvb", bufs=1, space="PSUM"))
        ps_kv1 = ctx.enter_context(tc.tile_pool(name="pskv1", bufs=1, space="PSUM"))
        ps_o = ctx.enter_context(tc.tile_pool(name="pso", bufs=2, space="PSUM"))

        ident = p_const.tile([P, P], bf16, tag="ident")
        make_identity(nc, ident[:])
        ones = p_const.tile([P, 1], bf16, tag="ones")
        nc.vector.memset(ones[:], 1.0)

        for _rep in range(n_reps):
            for pr in range(n_pairs):
                _emit_pair(
                    nc, mybir, f32, bf16, Alu, Act, T, pr,
                    p_qin, p_kin, p_vin, p_ek, p_rk, p_eq, p_phiqt, p_small,
                    p_out, ps_qt, ps_kva, ps_kvb, ps_kv1, ps_o,
                    q_d, k_d, v_d, o_d, ident, ones,
                )


def _emit_pair(
    nc, mybir, f32, bf16, Alu, Act, T, pr,
    p_qin, p_kin, p_vin, p_ek, p_rk, p_eq, p_phiqt, p_small, p_out,
    ps_qt, ps_kva, ps_kvb, ps_kv1, ps_o, q_d, k_d, v_d, o_d, ident, ones,
):
    iA, iB = 2 * pr, 2 * pr + 1
    D_ = 64

    # ---- loads: s = 32*p + t layout, SWDGE cast fp32 -> bf16 -------------
    q2 = p_qin.tile([P, 2, T, D_], bf16, tag="q2")
    k2 = p_kin.tile([P, 2, T, D_], f32, tag="k2")
    v2 = p_vin.tile([P, 2, T, D_], bf16, tag="v2")
    for h, i in ((0, iA), (1, iB)):
        nc.gpsimd.dma_start(q2[:, h], q_d[i].rearrange("(p t) d -> p t d", p=P))
        nc.sync.dma_start(k2[:, h], k_d[i].rearrange("(p t) d -> p t d", p=P))
        nc.gpsimd.dma_start(v2[:, h], v_d[i].rearrange("(p t) d -> p t d", p=P))

    # ---- k path: phi_k computed into the (t, h, d)-interleaved layout ----
    # ek[p, t, h, d]: (h, d) contiguous 128 so mm1's lhsT is a single free dim
    ek = p_ek.tile([P, T, 2, D_], bf16, tag="ek")
    rk = p_rk.tile([P, T, 2, D_], bf16, tag="rk")
    kch = max(T // 4, 1)
    for c0 in range(0, T, kch):
        sl = slice(c0, c0 + kch)
        kin = k2[:, :, sl, :].rearrange("p h t d -> p t h d")
        nc.scalar.activation(ek[:, sl], kin, Act.Exp, scale=SCALE)
        nc.vector.tensor_scalar(rk[:, sl], kin, 0.0, SCALE, Alu.max, Alu.mult)
        # phi_k = min(exp, 1) + relu, in place
        nc.vector.scalar_tensor_tensor(
            ek[:, sl], ek[:, sl], 1.0, rk[:, sl], Alu.min, Alu.add
        )

    # ---- q path: per-head PE transpose (bf16) -> phi_q transposed --------
    # phiqT[:, j, :]: partitions = (dA | dB), free = the 128 s of tile j
    phiqT = p_phiqt.tile([P, T, P], bf16, tag="phiqt")
    n_qb = (T + 3) // 4
    for jb in range(n_qb):
        njs = min(4, T - 4 * jb)
        qtp = ps_qt.tile([P, 4, P], bf16, tag="qtp")
        for jj in range(njs):
            j = 4 * jb + jj
            for h in (0, 1):
                nc.tensor.matmul(
                    qtp[64 * h : 64 * h + 64, jj, :],
                    q2[:, h, j, :],
                    ident[:],
                    is_transpose=True,
                )
        eq = p_eq.tile([P, 4, P], bf16, tag="eq")
        rq = p_eq.tile([P, 4, P], bf16, tag="rq")
        nc.scalar.activation(eq[:, :njs, :], qtp[:, :njs, :], Act.Exp, scale=SCALE)
        nc.scalar.activation(rq[:, :njs, :], qtp[:, :njs, :], Act.Relu, scale=SCALE)
        nc.vector.scalar_tensor_tensor(
            phiqT[:, 4 * jb : 4 * jb + njs, :],
            eq[:, :njs, :], 1.0, rq[:, :njs, :], Alu.min, Alu.add,
        )

    # ---- mm1: kv = phi_k^T @ v per head, k_one = phi_k^T @ 1 -------------
    # lhsT = ek[:, j] (both heads' (h,d) on its free dim -> out rows (h,d));
    # head h's kv block sits at out rows 64h..64h+63. Each accumulation
    # group owns a whole PSUM bank.
    kva = ps_kva.tile([P, D_], f32, tag="kva", name="kva")
    kvb = ps_kvb.tile([P, D_], f32, tag="kvb", name="kvb")
    kv1 = ps_kv1.tile([P, 1], f32, tag="kv1", name="kv1")
    for j in range(T):
        sta, sp = (j == 0), (j == T - 1)
        lhs = ek[:, j].rearrange("p h d -> p (h d)")
        nc.tensor.matmul(kva[:], lhs, v2[:, 0, j, :], start=sta, stop=sp)
        nc.tensor.matmul(kvb[:], lhs, v2[:, 1, j, :], start=sta, stop=sp)
        nc.tensor.matmul(kv1[:], lhs, ones[:], start=sta, stop=sp)

    # block-diagonal [128, 130]: rows 0-63 -> cols 0-64 (head A),
    # rows 64-127 -> cols 65-129 (head B); zeros elsewhere
    kvbd = p_small.tile([P, 130], bf16, tag="kvbd")
    nc.vector.memset(kvbd[:], 0.0)
    nc.vector.tensor_copy(out=kvbd[0:64, 0:64], in_=kva[0:64, :])
    nc.vector.tensor_copy(out=kvbd[0:64, 64:65], in_=kv1[0:64, :])
    nc.vector.tensor_copy(out=kvbd[64:128, 65:129], in_=kvb[64:128, :])
    nc.vector.tensor_copy(out=kvbd[64:128, 129:130], in_=kv1[64:128, :])

    # ---- mm2 + fused normalize ------------------------------------------
    out2 = p_out.tile([P, 2, T, D_], bf16, tag="out2")
    rc = p_small.tile([P, 2, T], f32, tag="recip")
    n_ob = (T + 2) // 3
    half_t = 18  # banks 0-5 cover t < 18
    for m in range(n_ob):
        w = min(3, T - 3 * m)
        op = ps_o.tile([P, 3, 130], f32, tag="op")
        for jj in range(w):
            j = 3 * m + jj
            # [128s, 130]: cols 0-64 head A (col 64 = norm), 65-129 head B
            nc.tensor.matmul(op[:, jj, :], phiqT[:, j, :], kvbd[:])
        opv = op[:, 0:w, :].rearrange("p j (h e) -> p j h e", h=2)
        nc.vector.reciprocal(
            rc[:, :, 3 * m : 3 * m + w],
            opv[:, :, :, 64].rearrange("p j h -> p h j"),
        )
        nc.vector.tensor_tensor(
            out2[:, :, 3 * m : 3 * m + w, :],
            opv[:, :, :, 0:64].rearrange("p j h e -> p h j e"),
            rc[:, :, 3 * m : 3 * m + w, None].to_broadcast((P, 2, w, D_)),
            Alu.mult,
        )
        if 3 * m + w == half_t:
            for h, i in ((0, iA), (1, iB)):
                od = o_d[i].rearrange("(p t) d -> p t d", p=P)
                nc.sync.dma_start(od[:, :half_t, :], out2[:, h, :half_t, :])

    # ---- store (second half) ----
    for h, i in ((0, iA), (1, iB)):
        od = o_d[i].rearrange("(p t) d -> p t d", p=P)
        nc.sync.dma_start(od[:, half_t:, :], out2[:, h, half_t:, :])


def _get_nc():
    key = (BH_PER_CORE, S_FULL)
    if key not in _NC_CACHE:
        _NC_CACHE[key] = build_bass(*key)
    return _NC_CACHE[key]


def run_sharded(q, k, v, trace=False):
    """q/k/v: [BH, S, D] fp32 numpy. Returns ([BH, S, D] fp32, BassKernelResults)."""
    from concourse.bass_utils import run_bass_kernel_spmd

    nc = _get_nc()
    in_maps = []
    for c in range(N_CORES):
        sl = slice(c * BH_PER_CORE, (c + 1) * BH_PER_CORE)
        in_maps.append(
            {
                "q": np.ascontiguousarray(q[sl]),
                "k": np.ascontiguousarray(k[sl]),
                "v": np.ascontiguousarray(v[sl]),
            }
        )
    res = run_bass_kernel_spmd(
        nc, in_maps, core_ids=list(range(N_CORES)), trace=trace
    )
    out = np.concatenate(
        [np.asarray(r["out"], dtype=np.float32) for r in res.results], axis=0
    )
    return out, res


def kernel(query, key, value, attention_mask=None):
    q = np.asarray(query, dtype=np.float32).reshape(BH, S_FULL, D)
    k = np.asarray(key, dtype=np.float32).reshape(BH, S_FULL, D)
    v = np.asarray(value, dtype=np.float32).reshape(BH, S_FULL, D)
    out, _ = run_sharded(q, k, v, trace=False)
    return out.reshape(B, H, S_FULL, D)


# revision 8
# speedup vs baseline: 2.7838x; 2.2594x over previous
"""Linear (feature-map) attention for Trainium2, 8-core head-parallel, bf16.

Math per (b,h), s = D**-0.25:
    phi(x) = elu(s*x) + 1  ==  min(exp(s*x), 1) + max(s*x, 0)
    kv     = phi_k^T @ [v | 1]            # [64, 65]; col 64 = sum_s phi_k
    out    = (phi_q @ kv[:, :64]) / (phi_q @ kv[:, 64])
(The reference's +1e-8 in the denominator is far below fp32 ulp of the
normalizer; the attention mask is all-ones per the input spec.)

All on-chip compute in bf16 (PE matmuls run 4x faster than fp32; rel-err
budget 2e-2 >> bf16 rounding). Inputs are cast fp32->bf16 during the DMA
load (SWDGE); output is computed fp32 from the fp32 PSUM accumulators.

Per core: 8 of the 64 (b,h) slices, processed as 4 pairs of heads.
SBUF s-layout: s = 32*p + t (p = partition, t = 0..31); every load moves
128 partitions x 4KB contiguous per head.

Engine plan (per-rep busy estimates from the instruction cost model):
  PE  : q-transpose per head via transpose-mode matmul (bf16) -> [dA|dB, s]
        mm1 col-sliced per head from interleaved phi_k (lhsT contiguous 128)
        mm2 full-K against block-diagonal kv -> [128s, 130] per s-tile
  ACT : Exp on k and q, Relu on q (Exp+Relu share one ACT table)
  Pool: Relu on k (tensor_scalar max+mult), SWDGE cast-DMA issue
  DVE : stt (min+add) assembling phi, kvbd assembly, reciprocal, normalize
"""

import numpy as np

B, H, S_FULL, D = 4, 16, 4096, 64
N_CORES = 8
BH = B * H
BH_PER_CORE = BH // N_CORES  # 8
P = 128

SCALE = float(D) ** -0.25          # 0.3535533905932738

_NC_CACHE = {}


def _patch_tile_drain():
    """The walrus build in this container accepts at most ONE sync wait per
    instruction, but TileContext's kernel-tail drain aggregates every
    outstanding semaphore onto a single SP Drain. Replace it with one
    single-wait SP nop per semaphore followed by the drain; likewise split
    any scheduled instruction that ends up with more than one sync wait."""
    import concourse.mybir as mybir
    import concourse.tile as tile
    from concourse.vector_clock import ScopedClock

    if getattr(tile.TileContext, "_single_wait_drain_patch", False):
        return

    def _drain_and_barrier(self, tick_clock, wait_clock):
        collector = self.nc.sync.nop()
        wait_clock.add_sem_waits(
            collector.ins, ScopedClock({None: tick_clock.global_clock})
        )
        waits = list(collector.ins.sync_info.on_wait) if collector.ins.sync_info else []
        collector.ins.sync_info = mybir.SyncInfo(on_wait=waits[:1], on_update=[])
        for w in waits[1:]:
            nop = self.nc.sync.nop()
            nop.ins.sync_info = mybir.SyncInfo(on_wait=[w], on_update=[])
        self.nc.sync.drain()
        self.nc.all_engine_barrier()
        assert self.sems is not None
        popped = self.nc._tile_sem_poison_stack.pop()
        assert popped is self._sem_poison
        self.nc.clear_and_free_semaphores(list(self.sems.allocated().values()))
        self.nc.all_engine_barrier()

    tile.TileContext._drain_and_barrier = _drain_and_barrier

    _orig_commit = tile.TileContext._commit_instruction

    def _commit_instruction(self, inst, lazy_reg_writes=True):
        si = getattr(inst, "sync_info", None)
        if si is not None and si.on_wait and len(si.on_wait) > 1:
            waits = list(si.on_wait)
            for w in waits[:-1]:
                nop = mybir.InstNoOp(
                    name=self.nc.get_next_instruction_name(),
                    engine=inst.engine,
                    text_hint="wait_split",
                    bass_nofuse=True,
                )
                nop.sync_info = mybir.SyncInfo(on_wait=[w], on_update=[])
                _orig_commit(self, nop, lazy_reg_writes)
            inst.sync_info = mybir.SyncInfo(
                on_wait=[waits[-1]], on_update=list(si.on_update or [])
            )
        return _orig_commit(self, inst, lazy_reg_writes)

    tile.TileContext._commit_instruction = _commit_instruction
    tile.TileContext._single_wait_drain_patch = True


def build_bass(n_heads=BH_PER_CORE, S=S_FULL, n_reps=1):
    import concourse.bass as bass
    import concourse.mybir as mybir
    import concourse.tile as tile

    _patch_tile_drain()

    f32 = mybir.dt.float32
    nc = bass.Bass("TRN2")
    q_d = nc.dram_tensor("q", [n_heads, S, D], f32, kind="ExternalInput")
    k_d = nc.dram_tensor("k", [n_heads, S, D], f32, kind="ExternalInput")
    v_d = nc.dram_tensor("v", [n_heads, S, D], f32, kind="ExternalInput")
    o_d = nc.dram_tensor(
        "out", [n_heads, S, D], mybir.dt.bfloat16, kind="ExternalOutput"
    )
    with tile.TileContext(nc) as tc:
        _emit(tc, q_d, k_d, v_d, o_d, n_heads, S, n_reps)
    nc.finalize()
    return nc


def _emit(tc, q_d, k_d, v_d, o_d, n_heads, S, n_reps=1):
    from contextlib import ExitStack

    import concourse.mybir as mybir
    from concourse.masks import make_identity

    nc = tc.nc
    f32 = mybir.dt.float32
    bf16 = mybir.dt.bfloat16
    Alu = mybir.AluOpType
    Act = mybir.ActivationFunctionType

    T = S // P                # s-tiles per head (32 for S=4096)
    n_pairs = n_heads // 2

    ctx = ExitStack()
    with ctx:
        p_const = ctx.enter_context(tc.tile_pool(name="const", bufs=1))
        p_qin = ctx.enter_context(tc.tile_pool(name="qin", bufs=2))
        p_kin = ctx.enter_context(tc.tile_pool(name="kin", bufs=2))
        p_vin = ctx.enter_context(tc.tile_pool(name="vin", bufs=2))
        p_ek = ctx.enter_context(tc.tile_pool(name="ek", bufs=2))
        p_rk = ctx.enter_context(tc.tile_pool(name="rk", bufs=2))
        p_eq = ctx.enter_context(tc.tile_pool(name="eq", bufs=2))
        p_phiqt = ctx.enter_context(tc.tile_pool(name="phiqt", bufs=2))
        p_small = ctx.enter_context(tc.tile_pool(name="small", bufs=2))
        p_out = ctx.enter_context(tc.tile_pool(name="outb", bufs=2))
        ps_qt = ctx.enter_context(tc.tile_pool(name="psqt", bufs=2, space="PSUM"))
        ps_kva = ctx.enter_context(tc.tile_pool(name="pskva", bufs=1, space="PSUM"))
        ps_kvb = ctx.enter_context(tc.tile_pool(name="pskvb", bufs=1, space="PSUM"))
        ps_kv1 = ctx.enter_context(tc.tile_pool(name="pskv1", bufs=1, space="PSUM"))
        ps_o = ctx.enter_context(tc.tile_pool(name="pso", bufs=2, space="PSUM"))

        ident = p_const.tile([P, P], bf16, tag="ident")
        make_identity(nc, ident[:])
        ones = p_const.tile([P, 1], bf16, tag="ones")
        nc.vector.memset(ones[:], 1.0)

        pending_store = None
        for _rep in range(n_reps):
            for pr in range(n_pairs):
                pending_store = _emit_pair(
                    nc, mybir, f32, bf16, Alu, Act, T, pr,
                    p_qin, p_kin, p_vin, p_ek, p_rk, p_eq, p_phiqt, p_small,
                    p_out, ps_qt, ps_kva, ps_kvb, ps_kv1, ps_o,
                    q_d, k_d, v_d, o_d, ident, ones, pending_store,
                )
        if pending_store is not None:
            pending_store()


def _emit_pair(
    nc, mybir, f32, bf16, Alu, Act, T, pr,
    p_qin, p_kin, p_vin, p_ek, p_rk, p_eq, p_phiqt, p_small, p_out,
    ps_qt, ps_kva, ps_kvb, ps_kv1, ps_o, q_d, k_d, v_d, o_d, ident, ones,
    pending_store=None,
):
    iA, iB = 2 * pr, 2 * pr + 1
    D_ = 64

    # ---- loads: s = 32*p + t layout, SWDGE cast fp32 -> bf16 -------------
    q2 = p_qin.tile([P, 2, T, D_], bf16, tag="q2")
    k2 = p_kin.tile([P, 2, T, D_], f32, tag="k2")
    v2 = p_vin.tile([P, 2, T, D_], bf16, tag="v2")
    for h, i in ((0, iA), (1, iB)):
        nc.gpsimd.dma_start(q2[:, h], q_d[i].rearrange("(p t) d -> p t d", p=P))
        nc.sync.dma_start(k2[:, h], k_d[i].rearrange("(p t) d -> p t d", p=P))
        nc.gpsimd.dma_start(v2[:, h], v_d[i].rearrange("(p t) d -> p t d", p=P))
    if pending_store is not None:
        pending_store()

    # ---- k path: phi_k computed into the (t, h, d)-interleaved layout ----
    # ek[p, t, h, d]: (h, d) contiguous 128 so mm1's lhsT is a single free dim
    ek = p_ek.tile([P, T, 2, D_], bf16, tag="ek")
    rk = p_rk.tile([P, T, 2, D_], bf16, tag="rk")
    kch = max(T // 4, 1)
    for c0 in range(0, T, kch):
        sl = slice(c0, c0 + kch)
        kin = k2[:, :, sl, :].rearrange("p h t d -> p t h d")
        nc.scalar.activation(ek[:, sl], kin, Act.Exp, scale=SCALE)
        nc.vector.tensor_scalar(rk[:, sl], kin, 0.0, SCALE, Alu.max, Alu.mult)
        # phi_k = min(exp, 1) + relu, in place
        nc.vector.scalar_tensor_tensor(
            ek[:, sl], ek[:, sl], 1.0, rk[:, sl], Alu.min, Alu.add
        )

    # ---- q path: per-head PE transpose (bf16) -> phi_q transposed --------
    # phiqT[:, j, :]: partitions = (dA | dB), free = the 128 s of tile j
    phiqT = p_phiqt.tile([P, T, P], bf16, tag="phiqt")
    n_qb = (T + 3) // 4
    for jb in range(n_qb):
        njs = min(4, T - 4 * jb)
        qtp = ps_qt.tile([P, 4, P], bf16, tag="qtp")
        for jj in range(njs):
            j = 4 * jb + jj
            for h in (0, 1):
                nc.tensor.matmul(
                    qtp[64 * h : 64 * h + 64, jj, :],
                    q2[:, h, j, :],
                    ident[:],
                    is_transpose=True,
                )
        eq = p_eq.tile([P, 4, P], bf16, tag="eq")
        rq = p_eq.tile([P, 4, P], bf16, tag="rq")
        nc.scalar.activation(eq[:, :njs, :], qtp[:, :njs, :], Act.Exp, scale=SCALE)
        nc.scalar.activation(rq[:, :njs, :], qtp[:, :njs, :], Act.Relu, scale=SCALE)
        nc.vector.scalar_tensor_tensor(
            phiqT[:, 4 * jb : 4 * jb + njs, :],
            eq[:, :njs, :], 1.0, rq[:, :njs, :], Alu.min, Alu.add,
        )

    # ---- mm1: kv = phi_k^T @ v per head, k_one = phi_k^T @ 1 -------------
    # lhsT = ek[:, j] (both heads' (h,d) on its free dim -> out rows (h,d));
    # head h's kv block sits at out rows 64h..64h+63. Each accumulation
    # group owns a whole PSUM bank.
    kva = ps_kva.tile([P, D_], f32, tag="kva", name="kva")
    kvb = ps_kvb.tile([P, D_], f32, tag="kvb", name="kvb")
    kv1 = ps_kv1.tile([P, 1], f32, tag="kv1", name="kv1")
    for j in range(T):
        sta, sp = (j == 0), (j == T - 1)
        lhs = ek[:, j].rearrange("p h d -> p (h d)")
        nc.tensor.matmul(kva[:], lhs, v2[:, 0, j, :], start=sta, stop=sp)
        nc.tensor.matmul(kvb[:], lhs, v2[:, 1, j, :], start=sta, stop=sp)
        nc.tensor.matmul(kv1[:], lhs, ones[:], start=sta, stop=sp)

    # block-diagonal [128, 130]: rows 0-63 -> cols 0-64 (head A),
    # rows 64-127 -> cols 65-129 (head B); zeros elsewhere
    kvbd = p_small.tile([P, 130], bf16, tag="kvbd")
    nc.vector.memset(kvbd[:], 0.0)
    nc.vector.tensor_copy(out=kvbd[0:64, 0:64], in_=kva[0:64, :])
    nc.vector.tensor_copy(out=kvbd[0:64, 64:65], in_=kv1[0:64, :])
    nc.vector.tensor_copy(out=kvbd[64:128, 65:129], in_=kvb[64:128, :])
    nc.vector.tensor_copy(out=kvbd[64:128, 129:130], in_=kv1[64:128, :])

    # ---- mm2 + fused normalize ------------------------------------------
    out2 = p_out.tile([P, 2, T, D_], bf16, tag="out2")
    rc = p_small.tile([P, 2, T], f32, tag="recip")
    n_ob = (T + 2) // 3
    for m in range(n_ob):
        w = min(3, T - 3 * m)
        op = ps_o.tile([P, 3, 130], f32, tag="op")
        for jj in range(w):
            j = 3 * m + jj
            # [128s, 130]: cols 0-64 head A (col 64 = norm), 65-129 head B
            nc.tensor.matmul(op[:, jj, :], phiqT[:, j, :], kvbd[:])
        opv = op[:, 0:w, :].rearrange("p j (h e) -> p j h e", h=2)
        nc.vector.reciprocal(
            rc[:, :, 3 * m : 3 * m + w],
            opv[:, :, :, 64].rearrange("p j h -> p h j"),
        )
        nc.vector.tensor_tensor(
            out2[:, :, 3 * m : 3 * m + w, :],
            opv[:, :, :, 0:64].rearrange("p j h e -> p h j e"),
            rc[:, :, 3 * m : 3 * m + w, None].to_broadcast((P, 2, w, D_)),
            Alu.mult,
        )
    # ---- store: deferred to after the NEXT pair's loads so the store's
    # sem wait never blocks upcoming k-loads on the SP stream ----
    def _store():
        for h, i in ((0, iA), (1, iB)):
            od = o_d[i].rearrange("(p t) d -> p t d", p=P)
            nc.sync.dma_start(od, out2[:, h])

    return _store


def _get_nc():
    key = (BH_PER_CORE, S_FULL)
    if key not in _NC_CACHE:
        _NC_CACHE[key] = build_bass(*key)
    return _NC_CACHE[key]


def run_sharded(q, k, v, trace=False):
    """q/k/v: [BH, S, D] fp32 numpy. Returns ([BH, S, D] fp32, BassKernelResults)."""
    from concourse.bass_utils import run_bass_kernel_spmd

    nc = _get_nc()
    in_maps = []
    for c in range(N_CORES):
        sl = slice(c * BH_PER_CORE, (c + 1) * BH_PER_CORE)
        in_maps.append(
            {
                "q": np.ascontiguousarray(q[sl]),
                "k": np.ascontiguousarray(k[sl]),
                "v": np.ascontiguousarray(v[sl]),
            }
        )
    res = run_bass_kernel_spmd(
        nc, in_maps, core_ids=list(range(N_CORES)), trace=trace
    )
    out = np.concatenate(
        [np.asarray(r["out"], dtype=np.float32) for r in res.results], axis=0
    )
    return out, res


def kernel(query, key, value, attention_mask=None):
    q = np.asarray(query, dtype=np.float32).reshape(BH, S_FULL, D)
    k = np.asarray(key, dtype=np.float32).reshape(BH, S_FULL, D)
    v = np.asarray(value, dtype=np.float32).reshape(BH, S_FULL, D)
    out, _ = run_sharded(q, k, v, trace=False)
    return out.reshape(B, H, S_FULL, D)
